# revision 1
# baseline (speedup 1.0000x reference)
"""JoinConvNet Trainium2 kernel — 8-core data-parallel, raw Bass.

Math per subnet (reference):
  conv(x, w)  : x[B,1,L,E], w[C,1,W,E], VALID -> c[B,C,L-W+1]
  m = max_l c ; h = relu(m + b_conv) ; o = relu(h @ w_fc.T + b_fc)
  out[b] = dot(o1[b], o2[b])

Device strategy (per core, 128 batches):
  Host pre-transposes x to X^T[E, B*L] and packs into 5 row-planes of <=128:
    plane0/1 = x1 e[0:128>, e[128:256>; plane2/3 = x2; plane4 = x1 e[256:300>
    at partitions 0:44, x2 e[256:300> at partitions 44:88, zeros elsewhere.
  Conv = 9 accumulating matmuls per 400-position chunk (3 taps x 3 e-chunks),
  tap shift folded into the rhs free-dim offset. PSUM [80,400] -> per-batch
  reduce_max -> H[80,128] -> bias+relu -> FC matmul -> bias+relu -> elementwise
  mul -> ones-matmul partition sum -> [1,128] out.

Conv matmuls run in float32r (fp32 bytes, full PE rate at N>=256).
"""
import os
import numpy as np
from contextlib import ExitStack

import concourse.bass as bass
import concourse.mybir as mybir
from concourse.bass_utils import run_bass_kernel_spmd

B, L, E = 1024, 200, 300
C, W, O = 80, 3, 30
NCORES = 8
BS = B // NCORES            # 128 batches/core
POS = BS * L                # 25600 positions/core
PADPOS = POS + 2
NSLAB = int(os.environ.get("K_NSLAB", "32"))
SLAB = POS // NSLAB         # positions per slab
SLABW = SLAB + 2            # loaded columns per slab
NG = SLAB // 400            # 400-position groups per slab
SLOTS = int(os.environ.get("K_SLOTS", "4"))
SPLIT_DMA = bool(int(os.environ.get("K_SPLIT_DMA", "0")))
GN = 400                    # matmul moving size
VALID = L - W + 1           # 198

CONV_DT = mybir.dt.float32r
F32 = mybir.dt.float32

LAST_RESULT = None
TRACE = bool(os.environ.get("KERNEL_TRACE"))
_NC_CACHE = {}


def _build_nc():
    nc = bass.Bass()
    xp = nc.declare_dram_parameter("xp", [5, 128, PADPOS], CONV_DT, isOutput=False)
    wst = nc.declare_dram_parameter("wst", [128, 18 * C], CONV_DT, isOutput=False)
    wf = nc.declare_dram_parameter("wf", [C, 2 * O], F32, isOutput=False)
    bc = nc.declare_dram_parameter("bc", [C, 2], F32, isOutput=False)
    bf = nc.declare_dram_parameter("bf", [O, 2], F32, isOutput=False)
    out = nc.declare_dram_parameter("out", [1, BS], F32, isOutput=True)

    with ExitStack() as ctx:
        X = ctx.enter_context(nc.sbuf_tensor([128, SLOTS, 5, SLABW], CONV_DT))
        Wc = ctx.enter_context(nc.sbuf_tensor([128, 18 * C], CONV_DT))
        Wf = ctx.enter_context(nc.sbuf_tensor([C, 2 * O], F32))
        Bc = ctx.enter_context(nc.sbuf_tensor([C, 2], F32))
        Bf = ctx.enter_context(nc.sbuf_tensor([O, 2], F32))
        ones = ctx.enter_context(nc.sbuf_tensor([O, 1], F32))
        H = ctx.enter_context(nc.sbuf_tensor([C, 2, BS], F32))
        Hr = ctx.enter_context(nc.sbuf_tensor([C, 2, BS], F32))
        Ofc = ctx.enter_context(nc.sbuf_tensor([O, 2, BS], F32))
        P = ctx.enter_context(nc.sbuf_tensor([O, BS], F32))
        osb = ctx.enter_context(nc.sbuf_tensor([1, BS], F32))
        cps = [ctx.enter_context(nc.psum_tensor(f"cps{i}", [C, GN], F32)) for i in range(4)]
        fps = [ctx.enter_context(nc.psum_tensor(f"fps{i}", [O, BS], F32)) for i in range(2)]
        dps = ctx.enter_context(nc.psum_tensor([1, BS], F32))

        dma_sem = ctx.enter_context(nc.semaphore("dma_sem"))
        pe_sem = ctx.enter_context(nc.semaphore("pe_sem"))
        red_sem = ctx.enter_context(nc.semaphore("red_sem"))
        act_sem = ctx.enter_context(nc.semaphore("act_sem"))
        fc_sem = ctx.enter_context(nc.semaphore("fc_sem"))
        block = ctx.enter_context(nc.Block())

        @block.sync
        def _(sync):
            sync.dma_start(out=Wc[:, :], in_=wst[:, :]).then_inc(dma_sem, 16)
            sync.dma_start(out=Wf[:, :], in_=wf[:, :]).then_inc(dma_sem, 16)
            sync.dma_start(out=Bc[:, :], in_=bc[:, :]).then_inc(dma_sem, 16)
            sync.dma_start(out=Bf[:, :], in_=bf[:, :]).then_inc(dma_sem, 16)
            for s in range(NSLAB):
                if s >= SLOTS:
                    # slot s%SLOTS free once PE finished slab s-SLOTS
                    sync.wait_ge(pe_sem, 2 * NG * (s - SLOTS + 1))
                for j in range(5):
                    if SPLIT_DMA and j >= 2:
                        continue
                    sync.dma_start(
                        out=X[:, s % SLOTS, j, :],
                        in_=xp[j, :, s * SLAB : s * SLAB + SLABW],
                    ).then_inc(dma_sem, 16)
            sync.wait_ge(act_sem, 3)
            sync.dma_start(out=out[:, :], in_=osb[:, :]).then_inc(dma_sem, 16)

        @block.tensor
        def _(tensor):
            k = 0
            for s in range(NSLAB):
                tensor.wait_ge(dma_sem, 64 + 5 * 16 * (s + 1))
                for g in range(NG):
                    for n in range(2):
                        if k >= 4:
                            tensor.wait_ge(red_sem, k - 3)
                        idx = 0
                        for w in range(W):
                            for j in range(3):
                                plane = (2 * n + j) if j < 2 else 4
                                col = (n * 9 + w * 3 + j) * C
                                mm = tensor.matmul(
                                    cps[k % 4][:, :],
                                    Wc[:, col : col + C],
                                    X[:, s % SLOTS, plane, g * GN + w : g * GN + w + GN],
                                    start=(idx == 0),
                                    stop=(idx == 8),
                                )
                                idx += 1
                        mm.then_inc(pe_sem, 1)
                        k += 1
            # FC + dot tail
            tensor.wait_ge(act_sem, 1)
            tensor.matmul(fps[0][:, :], Wf[:, 0:O], Hr[:, 0, :], start=True, stop=True)
            tensor.matmul(
                fps[1][:, :], Wf[:, O : 2 * O], Hr[:, 1, :], start=True, stop=True
            ).then_inc(fc_sem, 1)
            tensor.wait_ge(red_sem, 2 * NG * NSLAB + 1)
            tensor.matmul(dps[:, :], ones[:, :], P[:, :], start=True, stop=True).then_inc(
                fc_sem, 1
            )

        @block.vector
        def _(vector):
            vector.memset(ones[:, :], 1.0)
            k = 0
            for s in range(NSLAB):
                for g in range(NG):
                    p = s * NG + g  # batch pair index
                    for n in range(2):
                        vector.wait_ge(pe_sem, k + 1)
                        vector.reduce_max(
                            H[:, n, 2 * p : 2 * p + 1],
                            cps[k % 4][:, 0:VALID],
                            axis=mybir.AxisListType.X,
                        )
                        vector.reduce_max(
                            H[:, n, 2 * p + 1 : 2 * p + 2],
                            cps[k % 4][:, 200 : 200 + VALID],
                            axis=mybir.AxisListType.X,
                        ).then_inc(red_sem, 1)
                        k += 1
            vector.wait_ge(act_sem, 2)
            vector.tensor_mul(P[:, :], Ofc[:, 0, :], Ofc[:, 1, :]).then_inc(red_sem, 1)

        @block.scalar
        def _(scalar):
            if SPLIT_DMA:
                for s in range(NSLAB):
                    if s >= SLOTS:
                        scalar.wait_ge(pe_sem, 2 * NG * (s - SLOTS + 1))
                    for j in range(2, 5):
                        scalar.dma_start(
                            out=X[:, s % SLOTS, j, :],
                            in_=xp[j, :, s * SLAB : s * SLAB + SLABW],
                        ).then_inc(dma_sem, 16)
            scalar.wait_ge(red_sem, 2 * NG * NSLAB)
            scalar.activation(
                Hr[:, 0, :], H[:, 0, :], mybir.ActivationFunctionType.Relu,
                bias=Bc[:, 0:1],
            )
            scalar.activation(
                Hr[:, 1, :], H[:, 1, :], mybir.ActivationFunctionType.Relu,
                bias=Bc[:, 1:2],
            ).then_inc(act_sem, 1)
            scalar.wait_ge(fc_sem, 1)
            scalar.activation(
                Ofc[:, 0, :], fps[0][:, :], mybir.ActivationFunctionType.Relu,
                bias=Bf[:, 0:1],
            )
            scalar.activation(
                Ofc[:, 1, :], fps[1][:, :], mybir.ActivationFunctionType.Relu,
                bias=Bf[:, 1:2],
            ).then_inc(act_sem, 1)
            scalar.wait_ge(fc_sem, 2)
            scalar.copy(osb[:, :], dps[:, :]).then_inc(act_sem, 1)

    return nc


def _prep_weights(w_conv1, w_conv2, w_fc1, w_fc2, b_conv1, b_conv2, b_fc1, b_fc2):
    wst = np.zeros((128, 18, C), dtype=np.float32)
    for n, wc in enumerate((w_conv1, w_conv2)):
        wcs = wc[:, 0]  # [C, W, E]
        for w in range(W):
            for j in range(3):
                idx = n * 9 + w * 3 + j
                if j < 2:
                    wst[:, idx, :] = wcs[:, w, 128 * j : 128 * (j + 1)].T
                else:
                    sl = wcs[:, w, 256:300].T  # [44, C]
                    if n == 0:
                        wst[0:44, idx, :] = sl
                    else:
                        wst[44:88, idx, :] = sl
    wf = np.concatenate([w_fc1.T, w_fc2.T], axis=1).astype(np.float32)  # [C, 2O]
    bc = np.stack([b_conv1, b_conv2], axis=1).astype(np.float32)  # [C, 2]
    bf = np.stack([b_fc1, b_fc2], axis=1).astype(np.float32)  # [O, 2]
    return wst.reshape(128, 18 * C), wf, bc, bf


def kernel(x1, x2, w_conv1, b_conv1, w_fc1, b_fc1, w_conv2, b_conv2, w_fc2, b_fc2):
    global LAST_RESULT
    x1 = np.ascontiguousarray(np.asarray(x1, dtype=np.float32))
    x2 = np.ascontiguousarray(np.asarray(x2, dtype=np.float32))
    wst, wf, bc, bf = _prep_weights(
        np.asarray(w_conv1, np.float32), np.asarray(w_conv2, np.float32),
        np.asarray(w_fc1, np.float32), np.asarray(w_fc2, np.float32),
        np.asarray(b_conv1, np.float32), np.asarray(b_conv2, np.float32),
        np.asarray(b_fc1, np.float32), np.asarray(b_fc2, np.float32),
    )

    if "nc" not in _NC_CACHE:
        _NC_CACHE["nc"] = _build_nc()
    nc = _NC_CACHE["nc"]

    in_maps = []
    for c in range(NCORES):
        xs1 = x1[c * BS : (c + 1) * BS, 0].reshape(POS, E).T  # [300, POS]
        xs2 = x2[c * BS : (c + 1) * BS, 0].reshape(POS, E).T
        xp = np.zeros((5, 128, PADPOS), dtype=np.float32)
        xp[0, :, :POS] = xs1[0:128]
        xp[1, :, :POS] = xs1[128:256]
        xp[2, :, :POS] = xs2[0:128]
        xp[3, :, :POS] = xs2[128:256]
        xp[4, 0:44, :POS] = xs1[256:300]
        xp[4, 44:88, :POS] = xs2[256:300]
        in_maps.append({"xp": xp, "wst": wst, "wf": wf, "bc": bc, "bf": bf})

    res = run_bass_kernel_spmd(nc, in_maps, list(range(NCORES)), trace=TRACE)
    LAST_RESULT = res
    return np.concatenate(
        [res.results[c]["out"].reshape(BS, 1) for c in range(NCORES)], axis=0
    )



# revision 37
# speedup vs baseline: 2.6065x; 2.6065x over previous
"""JoinConvNet Trainium2 kernel — 8-core data-parallel, raw Bass, fp8 DoubleRow.

Math per subnet (reference):
  conv(x, w)  : x[B,1,L,E], w[C,1,W,E], VALID -> c[B,C,L-W+1]
  m = max_l c ; h = relu(m + b_conv) ; o = relu(h @ w_fc.T + b_fc)
  out[b] = dot(o1[b], o2[b])

Quantization scheme (rel err ~1.74e-2 < 2e-2 gate, validated vs reference):
  x scaled x8 -> fp8 e4m3. conv weights scaled x64 -> hi = e4m3(w), lo =
  e4m3(w - hi); the e[0:256] chunks use hi+lo (bf16-class accuracy), the
  e[256:300] remainder uses hi only. psum carries 512*conv; bias scaled
  x512 into the relu, FC weights divided by 512 (bf16).

Device layout (per core, 128 batches, POS=25600 positions):
  Host packs x^T into 7 row-planes of [128, PADPOS] fp8:
    0: x1 e[0:128)   1: x1 e[128:256)  2: x2 e[0:128)  3: x2 e[128:256)
    4: mega1 = x1 e[256:300) pre-shifted per tap: rows [t0(44) t1(44) t2(40)]
    5: ovf   = rows 0:4 x1 (t2, e296:300), rows 4:8 x2 (t2, e296:300)
    6: mega2 = x2 remainder, like mega1
  Conv = 7 accumulating fp8 DoubleRow matmuls per (group=400 cols, subnet):
    d=0..5: (tap t, hi/lo h): ktiles = (L, H) planes at col offset +t
    d=6:    ktiles = (mega_n, ovf) - remainder taps baked into plane rows
  Each DoubleRow costs out_free(400) * 0.5 cycles - 2 k-tiles per matmul.
  PSUM [80,2,200] per (group parity, subnet) -> DVE reduce_max [80,2,198]
  -> H -> bias+relu (bf16) -> FC matmul (bf16) -> bias+relu -> mul ->
  ones-matmul partition sum -> [1,128] out.
"""
import os
import numpy as np
import ml_dtypes
from contextlib import ExitStack

import concourse.bass as bass
import concourse.mybir as mybir
from concourse.bass_utils import run_bass_kernel_spmd

B, L, E = 1024, 200, 300
C, W, O = 80, 3, 30
NCORES = 8
BS = B // NCORES            # 128 batches/core
POS = BS * L                # 25600 positions/core
PADPOS = POS + 2
NSLAB = int(os.environ.get("K_NSLAB", "32"))
SLAB = POS // NSLAB         # positions per slab
SLABW = SLAB + 2            # loaded columns per slab
SLOTS = int(os.environ.get("K_SLOTS", "4"))
GN = 400                    # matmul moving size (2 samples)
PPS = SLAB // GN            # group pairs per slab
NPAIR = POS // GN           # 64 groups total
VALID = L - W + 1           # 198
NDR = 7                     # DoubleRow matmuls per (group, subnet)
ALT = bool(int(os.environ.get("K_ALT", "0")))  # even groups: 6-DR (tap2 fp8)
XS, WS = 8.0, 64.0          # x / w quantization scales

F8 = mybir.dt.float8e4
BF16 = mybir.dt.bfloat16
F32 = mybir.dt.float32
NP_F8 = ml_dtypes.float8_e4m3fn
NP_BF = ml_dtypes.bfloat16
DR = mybir.MatmulPerfMode.DoubleRow

LAST_RESULT = None
TRACE = bool(os.environ.get("KERNEL_TRACE"))
_NC_CACHE = {}


def _build_nc():
    nc = bass.Bass()
    xp = nc.declare_dram_parameter("xp", [7, 128, PADPOS], F8, isOutput=False)
    wst = nc.declare_dram_parameter("wst", [128, 2 * NDR + 2, 2, C], F8, isOutput=False)
    wfb = nc.declare_dram_parameter("wfb", [C, 2, O], BF16, isOutput=False)
    bc = nc.declare_dram_parameter("bc", [C, 2], F32, isOutput=False)
    bf = nc.declare_dram_parameter("bf", [O, 2], F32, isOutput=False)
    out = nc.declare_dram_parameter("out", [1, BS], F32, isOutput=True)

    with ExitStack() as ctx:
        X = ctx.enter_context(nc.sbuf_tensor([128, SLOTS, 7, SLABW], F8))
        Wc = ctx.enter_context(nc.sbuf_tensor([128, 2 * NDR + 2, 2, C], F8))
        Wf = ctx.enter_context(nc.sbuf_tensor([C, 2, O], BF16))
        Bc = ctx.enter_context(nc.sbuf_tensor([C, 2], F32))
        Bf = ctx.enter_context(nc.sbuf_tensor([O, 2], F32))
        ones = ctx.enter_context(nc.sbuf_tensor([O, 1], F32))
        H = ctx.enter_context(nc.sbuf_tensor([C, 2, BS], F32))
        Hr = ctx.enter_context(nc.sbuf_tensor([C, 2, BS], BF16))
        Ofc = ctx.enter_context(nc.sbuf_tensor([O, 2, BS], F32))
        P = ctx.enter_context(nc.sbuf_tensor([O, BS], F32))
        osb = ctx.enter_context(nc.sbuf_tensor([1, BS], F32))
        # psum accumulators: rotation over chains k = 2*g + n
        NPC = int(os.environ.get("K_NPC", "5"))
        cps = [ctx.enter_context(nc.psum_tensor(f"cps{i}", [C, 2, L], F32))
               for i in range(NPC)]
        NO_RWAIT = bool(int(os.environ.get("K_NO_RWAIT", "0")))
        NO_DWAIT = bool(int(os.environ.get("K_NO_DWAIT", "0")))
        fps = [ctx.enter_context(nc.psum_tensor(f"fps{n}", [O, BS], F32))
               for n in range(2)]
        dps = ctx.enter_context(nc.psum_tensor([1, BS], F32))

        dma_sem = ctx.enter_context(nc.semaphore("dma_sem"))
        pe_sem = ctx.enter_context(nc.semaphore("pe_sem"))
        red_sem = ctx.enter_context(nc.semaphore("red_sem"))
        act_sem = ctx.enter_context(nc.semaphore("act_sem"))
        fc_sem = ctx.enter_context(nc.semaphore("fc_sem"))
        block = ctx.enter_context(nc.Block())

        # Planes: [x1L, x1H, x2L, x2H, mega1, mega2, ovf]. d=0..5: planes
        # (2n, 2n+1) at col offset +tap; d=6: subnet0 -> (4,6) step 2,
        # subnet1 -> (5,6); ktile0 = mega_n, ktile1 = ovf (rows 4n:4n+4).

        @block.sync
        def _(sync):
            def slab_dma(sync, s):
                c0 = s * SLAB
                sync.dma_start(
                    out=X[:, s % SLOTS, :, :],
                    in_=xp[:, :, c0 : c0 + SLABW].transpose([1, 0, 2]),
                ).then_inc(dma_sem, 16)

            sync.dma_start(out=Wc[:, :, :, :], in_=wst[:, :, :, :]).then_inc(dma_sem, 16)
            slab_dma(sync, 0)
            for s in range(1, NSLAB):
                if s >= SLOTS:
                    # slot free once PE finished all pairs of slab s-SLOTS
                    sync.wait_ge(pe_sem, 2 * PPS * (s - SLOTS + 1))
                slab_dma(sync, s)
            sync.dma_start(out=Wf[:, :, :], in_=wfb[:, :, :]).then_inc(dma_sem, 16)
            sync.dma_start(out=Bc[:, :], in_=bc[:, :]).then_inc(dma_sem, 16)
            sync.dma_start(out=Bf[:, :], in_=bf[:, :]).then_inc(dma_sem, 16)
            sync.wait_ge(act_sem, 3)
            sync.dma_start(out=out[:, :], in_=osb[:, :]).then_inc(dma_sem, 16)

        @block.tensor
        def _(tensor):
            # p-state warmups: garbage DoubleRows hidden under the DMA prologue
            for _ in range(int(os.environ.get("K_WARM", "24"))):
                tensor.matmul(
                    cps[NPC - 1][:, :, :], Wc[:, 0],
                    X[:, 0, 0:2, 0:GN], start=True, stop=True, perf_mode=DR,
                )
            for s in range(NSLAB):
                if not NO_DWAIT:
                    # wst + slabs 0..s
                    tensor.wait_ge(dma_sem, 16 * (s + 2))
                for gl in range(PPS):
                    g = s * PPS + gl
                    c0 = gl * GN
                    for n in range(2):
                        k = 2 * g + n
                        if k >= NPC and not NO_RWAIT:
                            tensor.wait_ge(red_sem, k - NPC + 1)
                        if ALT and g % 2 == 0:
                            # 6-DR variant: tap2 L/H as one fp8 pair
                            dlist = [0, 1, 2, 3, 2 * NDR + n, 6]
                        else:
                            dlist = [0, 1, 2, 3, 4, 5, 6]
                        mm = None
                        for di, d in enumerate(dlist):
                            if d >= 2 * NDR:          # fp8 tap2 pair block
                                bi = d
                                p0, pstep, cc = 2 * n, 1, c0 + 2
                            elif d < 6:
                                t, _h = divmod(d, 2)
                                bi = NDR * n + d
                                p0, pstep, cc = 2 * n, 1, c0 + t
                            else:
                                bi = NDR * n + d
                                p0, pstep, cc = 4 + n, 2 - n, c0
                            mm = tensor.matmul(
                                cps[k % NPC][:, :, :],
                                Wc[:, bi],
                                X[:, s % SLOTS, p0 : p0 + pstep + 1 : pstep, cc : cc + GN],
                                start=(di == 0),
                                stop=(di == len(dlist) - 1),
                                perf_mode=DR,
                            )
                        mm.then_inc(pe_sem, 1)
            # FC + dot tail
            tensor.wait_ge(act_sem, 1)
            tensor.matmul(fps[0][:, :], Wf[:, 0], Hr[:, 0, :], start=True, stop=True)
            tensor.matmul(
                fps[1][:, :], Wf[:, 1], Hr[:, 1, :], start=True, stop=True
            ).then_inc(fc_sem, 1)
            tensor.wait_ge(red_sem, 2 * NPAIR + 1)
            tensor.matmul(dps[:, :], ones[:, :], P[:, :], start=True, stop=True).then_inc(
                fc_sem, 1
            )

        @block.vector
        def _(vector):
            vector.memset(ones[:, :], 1.0)
            for g in range(NPAIR):
                for n in range(2):
                    k = 2 * g + n
                    vector.wait_ge(pe_sem, k + 1)
                    vector.reduce_max(
                        H[:, n, 2 * g : 2 * g + 2],
                        cps[k % NPC][:, :, 0:VALID],
                        axis=mybir.AxisListType.X,
                    ).then_inc(red_sem, 1)
            vector.wait_ge(act_sem, 2)
            vector.tensor_mul(P[:, :], Ofc[:, 0, :], Ofc[:, 1, :]).then_inc(red_sem, 1)

        @block.scalar
        def _(scalar):
            scalar.wait_ge(dma_sem, 16 * (NSLAB + 4))
            scalar.wait_ge(red_sem, 2 * NPAIR)
            scalar.activation(
                Hr[:, 0, :], H[:, 0, :], mybir.ActivationFunctionType.Relu,
                bias=Bc[:, 0:1],
            )
            scalar.activation(
                Hr[:, 1, :], H[:, 1, :], mybir.ActivationFunctionType.Relu,
                bias=Bc[:, 1:2],
            ).then_inc(act_sem, 1)
            scalar.wait_ge(fc_sem, 1)
            scalar.activation(
                Ofc[:, 0, :], fps[0][:, :], mybir.ActivationFunctionType.Relu,
                bias=Bf[:, 0:1],
            )
            scalar.activation(
                Ofc[:, 1, :], fps[1][:, :], mybir.ActivationFunctionType.Relu,
                bias=Bf[:, 1:2],
            ).then_inc(act_sem, 1)
            scalar.wait_ge(fc_sem, 2)
            scalar.copy(osb[:, :], dps[:, :]).then_inc(act_sem, 1)


    return nc


def _prep_weights(w_conv1, w_conv2, w_fc1, w_fc2, b_conv1, b_conv2, b_fc1, b_fc2):
    # wst [128, 2*NDR+2, 2, C] fp8: DR block (n, d) at [:, NDR*n + d];
    # blocks 14+n: (L|H)@tap2 fp8-only pair for the alternating 6-DR groups
    wst = np.zeros((128, 2 * NDR + 2, 2, C), dtype=NP_F8)
    for n, wc in enumerate((w_conv1, w_conv2)):
        w = np.asarray(wc[:, 0], np.float32) * WS          # [C, W, E]
        whi = w.astype(NP_F8).astype(np.float32)
        wlo = (w - whi).astype(NP_F8)                      # residual, same scale
        whi8 = w.astype(NP_F8)
        for t in range(W):
            for h, wsrc in enumerate((whi8, wlo)):
                bi = NDR * n + 2 * t + h
                wst[:, bi, 0, :] = wsrc[:, t, 0:128].T
                wst[:, bi, 1, :] = wsrc[:, t, 128:256].T
        # d=6: remainder mega + ovf, hi-only weights
        mega = np.zeros((128, C), dtype=NP_F8)
        mega[0:44] = whi8[:, 0, 256:300].T
        mega[44:88] = whi8[:, 1, 256:300].T
        mega[88:128] = whi8[:, 2, 256:296].T
        ovf = np.zeros((128, C), dtype=NP_F8)
        ovf[4 * n : 4 * n + 4] = whi8[:, 2, 296:300].T
        bi = NDR * n + 6
        wst[:, bi, 0, :] = mega              # ktile0 = mega_n
        wst[:, bi, 1, :] = ovf               # ktile1 = ovf plane (6)
        wst[:, 2 * NDR + n, 0, :] = whi8[:, 2, 0:128].T     # fp8 tap2 L
        wst[:, 2 * NDR + n, 1, :] = whi8[:, 2, 128:256].T   # fp8 tap2 H
    sc = np.float32(1.0 / (XS * WS))
    wfb = np.stack([w_fc1.T * sc, w_fc2.T * sc], axis=1).astype(NP_BF)  # [C, 2, O]
    bcs = np.stack([b_conv1, b_conv2], axis=1).astype(np.float32) * np.float32(XS * WS)
    bfs = np.stack([b_fc1, b_fc2], axis=1).astype(np.float32)
    return wst, wfb, bcs, bfs


def _prep_x_core(xq1, xq2):
    # xq1/xq2: fp8 [BS, L, E] for this core -> xp [7, 128, PADPOS]
    xs1 = np.ascontiguousarray(xq1.reshape(POS, E).T)      # [300, POS] fp8
    xs2 = np.ascontiguousarray(xq2.reshape(POS, E).T)
    xp = np.zeros((7, 128, PADPOS), dtype=NP_F8)
    xp[0, :, :POS] = xs1[0:128]
    xp[1, :, :POS] = xs1[128:256]
    xp[2, :, :POS] = xs2[0:128]
    xp[3, :, :POS] = xs2[128:256]
    for mi, xs in ((4, xs1), (5, xs2)):
        # mega rows: taps baked via pre-shift
        xp[mi, 0:44, :POS] = xs[256:300]
        xp[mi, 44:88, : POS - 1] = xs[256:300, 1:]
        xp[mi, 88:128, : POS - 2] = xs[256:296, 2:]
    xp[6, 0:4, : POS - 2] = xs1[296:300, 2:]
    xp[6, 4:8, : POS - 2] = xs2[296:300, 2:]
    return xp


def kernel(x1, x2, w_conv1, b_conv1, w_fc1, b_fc1, w_conv2, b_conv2, w_fc2, b_fc2):
    global LAST_RESULT
    xq1 = (np.asarray(x1, np.float32)[:, 0] * np.float32(XS)).astype(NP_F8)
    xq2 = (np.asarray(x2, np.float32)[:, 0] * np.float32(XS)).astype(NP_F8)
    wst, wfb, bcs, bfs = _prep_weights(
        np.asarray(w_conv1, np.float32), np.asarray(w_conv2, np.float32),
        np.asarray(w_fc1, np.float32), np.asarray(w_fc2, np.float32),
        np.asarray(b_conv1, np.float32), np.asarray(b_conv2, np.float32),
        np.asarray(b_fc1, np.float32), np.asarray(b_fc2, np.float32),
    )

    if "nc" not in _NC_CACHE:
        _NC_CACHE["nc"] = _build_nc()
    nc = _NC_CACHE["nc"]

    in_maps = []
    for c in range(NCORES):
        xp = _prep_x_core(xq1[c * BS : (c + 1) * BS], xq2[c * BS : (c + 1) * BS])
        in_maps.append({"xp": xp, "wst": wst, "wfb": wfb, "bc": bcs, "bf": bfs})

    res = run_bass_kernel_spmd(nc, in_maps, list(range(NCORES)), trace=TRACE)
    LAST_RESULT = res
    return np.concatenate(
        [res.results[c]["out"].reshape(BS, 1) for c in range(NCORES)], axis=0
    )


# revision 38
# speedup vs baseline: 2.7759x; 1.0650x over previous
"""JoinConvNet Trainium2 kernel — 8-core data-parallel, raw Bass, fp8 DoubleRow.

Math per subnet (reference):
  conv(x, w)  : x[B,1,L,E], w[C,1,W,E], VALID -> c[B,C,L-W+1]
  m = max_l c ; h = relu(m + b_conv) ; o = relu(h @ w_fc.T + b_fc)
  out[b] = dot(o1[b], o2[b])

Quantization scheme (rel err ~1.74e-2 < 2e-2 gate, validated vs reference):
  x scaled x8 -> fp8 e4m3. conv weights scaled x64 -> hi = e4m3(w), lo =
  e4m3(w - hi); the e[0:256] chunks use hi+lo (bf16-class accuracy), the
  e[256:300] remainder uses hi only. psum carries 512*conv; bias scaled
  x512 into the relu, FC weights divided by 512 (bf16).

Device layout (per core, 128 batches, POS=25600 positions):
  Host packs x^T into 7 row-planes of [128, PADPOS] fp8:
    0: x1 e[0:128)   1: x1 e[128:256)  2: x2 e[0:128)  3: x2 e[128:256)
    4: mega1 = x1 e[256:300) pre-shifted per tap: rows [t0(44) t1(44) t2(40)]
    5: ovf   = rows 0:4 x1 (t2, e296:300), rows 4:8 x2 (t2, e296:300)
    6: mega2 = x2 remainder, like mega1
  Conv = 7 accumulating fp8 DoubleRow matmuls per (group=400 cols, subnet):
    d=0..5: (tap t, hi/lo h): ktiles = (L, H) planes at col offset +t
    d=6:    ktiles = (mega_n, ovf) - remainder taps baked into plane rows
  Each DoubleRow costs out_free(400) * 0.5 cycles - 2 k-tiles per matmul.
  PSUM [80,2,200] per (group parity, subnet) -> DVE reduce_max [80,2,198]
  -> H -> bias+relu (bf16) -> FC matmul (bf16) -> bias+relu -> mul ->
  ones-matmul partition sum -> [1,128] out.
"""
import os
import numpy as np
import ml_dtypes
from contextlib import ExitStack

import concourse.bass as bass
import concourse.mybir as mybir
from concourse.bass_utils import run_bass_kernel_spmd

B, L, E = 1024, 200, 300
C, W, O = 80, 3, 30
NCORES = 8
BS = B // NCORES            # 128 batches/core
POS = BS * L                # 25600 positions/core
PADPOS = POS + 2
NSLAB = int(os.environ.get("K_NSLAB", "32"))
SLAB = POS // NSLAB         # positions per slab
SLABW = SLAB + 2            # loaded columns per slab
SLOTS = int(os.environ.get("K_SLOTS", "4"))
GN = 400                    # matmul moving size (2 samples)
PPS = SLAB // GN            # group pairs per slab
NPAIR = POS // GN           # 64 groups total
VALID = L - W + 1           # 198
NDR = 7                     # DoubleRow matmuls per (group, subnet)
ALT = bool(int(os.environ.get("K_ALT", "1")))  # even groups: 6-DR (tap2 fp8)
XS, WS = 8.0, 64.0          # x / w quantization scales

F8 = mybir.dt.float8e4
BF16 = mybir.dt.bfloat16
F32 = mybir.dt.float32
NP_F8 = ml_dtypes.float8_e4m3fn
NP_BF = ml_dtypes.bfloat16
DR = mybir.MatmulPerfMode.DoubleRow

LAST_RESULT = None
TRACE = bool(os.environ.get("KERNEL_TRACE"))
_NC_CACHE = {}


def _build_nc():
    nc = bass.Bass()
    xp = nc.declare_dram_parameter("xp", [7, 128, PADPOS], F8, isOutput=False)
    wst = nc.declare_dram_parameter("wst", [128, 2 * NDR + 2, 2, C], F8, isOutput=False)
    wfb = nc.declare_dram_parameter("wfb", [C, 2, O], BF16, isOutput=False)
    bc = nc.declare_dram_parameter("bc", [C, 2], F32, isOutput=False)
    bf = nc.declare_dram_parameter("bf", [O, 2], F32, isOutput=False)
    out = nc.declare_dram_parameter("out", [1, BS], F32, isOutput=True)

    with ExitStack() as ctx:
        X = ctx.enter_context(nc.sbuf_tensor([128, SLOTS, 7, SLABW], F8))
        Wc = ctx.enter_context(nc.sbuf_tensor([128, 2 * NDR + 2, 2, C], F8))
        Wf = ctx.enter_context(nc.sbuf_tensor([C, 2, O], BF16))
        Bc = ctx.enter_context(nc.sbuf_tensor([C, 2], F32))
        Bf = ctx.enter_context(nc.sbuf_tensor([O, 2], F32))
        ones = ctx.enter_context(nc.sbuf_tensor([O, 1], F32))
        H = ctx.enter_context(nc.sbuf_tensor([C, 2, BS], F32))
        Hr = ctx.enter_context(nc.sbuf_tensor([C, 2, BS], BF16))
        Ofc = ctx.enter_context(nc.sbuf_tensor([O, 2, BS], F32))
        P = ctx.enter_context(nc.sbuf_tensor([O, BS], F32))
        osb = ctx.enter_context(nc.sbuf_tensor([1, BS], F32))
        # psum accumulators: rotation over chains k = 2*g + n
        NPC = int(os.environ.get("K_NPC", "5"))
        cps = [ctx.enter_context(nc.psum_tensor(f"cps{i}", [C, 2, L], F32))
               for i in range(NPC)]
        NO_RWAIT = bool(int(os.environ.get("K_NO_RWAIT", "0")))
        NO_DWAIT = bool(int(os.environ.get("K_NO_DWAIT", "0")))
        fps = [ctx.enter_context(nc.psum_tensor(f"fps{n}", [O, BS], F32))
               for n in range(2)]
        dps = ctx.enter_context(nc.psum_tensor([1, BS], F32))

        dma_sem = ctx.enter_context(nc.semaphore("dma_sem"))
        pe_sem = ctx.enter_context(nc.semaphore("pe_sem"))
        red_sem = ctx.enter_context(nc.semaphore("red_sem"))
        act_sem = ctx.enter_context(nc.semaphore("act_sem"))
        fc_sem = ctx.enter_context(nc.semaphore("fc_sem"))
        block = ctx.enter_context(nc.Block())

        # Planes: [x1L, x1H, x2L, x2H, mega1, mega2, ovf]. d=0..5: planes
        # (2n, 2n+1) at col offset +tap; d=6: subnet0 -> (4,6) step 2,
        # subnet1 -> (5,6); ktile0 = mega_n, ktile1 = ovf (rows 4n:4n+4).

        @block.sync
        def _(sync):
            def slab_dma(sync, s):
                c0 = s * SLAB
                sync.dma_start(
                    out=X[:, s % SLOTS, :, :],
                    in_=xp[:, :, c0 : c0 + SLABW].transpose([1, 0, 2]),
                ).then_inc(dma_sem, 16)

            sync.dma_start(out=Wc[:, :, :, :], in_=wst[:, :, :, :]).then_inc(dma_sem, 16)
            slab_dma(sync, 0)
            for s in range(1, NSLAB):
                if s >= SLOTS:
                    # slot free once PE finished all pairs of slab s-SLOTS
                    sync.wait_ge(pe_sem, 2 * PPS * (s - SLOTS + 1))
                slab_dma(sync, s)
            sync.dma_start(out=Wf[:, :, :], in_=wfb[:, :, :]).then_inc(dma_sem, 16)
            sync.dma_start(out=Bc[:, :], in_=bc[:, :]).then_inc(dma_sem, 16)
            sync.dma_start(out=Bf[:, :], in_=bf[:, :]).then_inc(dma_sem, 16)
            sync.wait_ge(act_sem, 3)
            sync.dma_start(out=out[:, :], in_=osb[:, :]).then_inc(dma_sem, 16)

        @block.tensor
        def _(tensor):
            # p-state warmups: garbage DoubleRows hidden under the DMA prologue
            for _ in range(int(os.environ.get("K_WARM", "24"))):
                tensor.matmul(
                    cps[NPC - 1][:, :, :], Wc[:, 0],
                    X[:, 0, 0:2, 0:GN], start=True, stop=True, perf_mode=DR,
                )
            for s in range(NSLAB):
                if not NO_DWAIT:
                    # wst + slabs 0..s
                    tensor.wait_ge(dma_sem, 16 * (s + 2))
                for gl in range(PPS):
                    g = s * PPS + gl
                    c0 = gl * GN
                    for n in range(2):
                        k = 2 * g + n
                        if k >= NPC and not NO_RWAIT:
                            tensor.wait_ge(red_sem, k - NPC + 1)
                        if ALT and g % 2 == 0:
                            # 6-DR variant: tap2 L/H as one fp8 pair
                            dlist = [0, 1, 2, 3, 2 * NDR + n, 6]
                        else:
                            dlist = [0, 1, 2, 3, 4, 5, 6]
                        mm = None
                        for di, d in enumerate(dlist):
                            if d >= 2 * NDR:          # fp8 tap2 pair block
                                bi = d
                                p0, pstep, cc = 2 * n, 1, c0 + 2
                            elif d < 6:
                                t, _h = divmod(d, 2)
                                bi = NDR * n + d
                                p0, pstep, cc = 2 * n, 1, c0 + t
                            else:
                                bi = NDR * n + d
                                p0, pstep, cc = 4 + n, 2 - n, c0
                            mm = tensor.matmul(
                                cps[k % NPC][:, :, :],
                                Wc[:, bi],
                                X[:, s % SLOTS, p0 : p0 + pstep + 1 : pstep, cc : cc + GN],
                                start=(di == 0),
                                stop=(di == len(dlist) - 1),
                                perf_mode=DR,
                            )
                        mm.then_inc(pe_sem, 1)
            # FC + dot tail
            tensor.wait_ge(act_sem, 1)
            tensor.matmul(fps[0][:, :], Wf[:, 0], Hr[:, 0, :], start=True, stop=True)
            tensor.matmul(
                fps[1][:, :], Wf[:, 1], Hr[:, 1, :], start=True, stop=True
            ).then_inc(fc_sem, 1)
            tensor.wait_ge(red_sem, 2 * NPAIR + 1)
            tensor.matmul(dps[:, :], ones[:, :], P[:, :], start=True, stop=True).then_inc(
                fc_sem, 1
            )

        @block.vector
        def _(vector):
            vector.memset(ones[:, :], 1.0)
            for g in range(NPAIR):
                for n in range(2):
                    k = 2 * g + n
                    vector.wait_ge(pe_sem, k + 1)
                    vector.reduce_max(
                        H[:, n, 2 * g : 2 * g + 2],
                        cps[k % NPC][:, :, 0:VALID],
                        axis=mybir.AxisListType.X,
                    ).then_inc(red_sem, 1)
            vector.wait_ge(act_sem, 2)
            vector.tensor_mul(P[:, :], Ofc[:, 0, :], Ofc[:, 1, :]).then_inc(red_sem, 1)

        @block.scalar
        def _(scalar):
            scalar.wait_ge(dma_sem, 16 * (NSLAB + 4))
            scalar.wait_ge(red_sem, 2 * NPAIR)
            scalar.activation(
                Hr[:, 0, :], H[:, 0, :], mybir.ActivationFunctionType.Relu,
                bias=Bc[:, 0:1],
            )
            scalar.activation(
                Hr[:, 1, :], H[:, 1, :], mybir.ActivationFunctionType.Relu,
                bias=Bc[:, 1:2],
            ).then_inc(act_sem, 1)
            scalar.wait_ge(fc_sem, 1)
            scalar.activation(
                Ofc[:, 0, :], fps[0][:, :], mybir.ActivationFunctionType.Relu,
                bias=Bf[:, 0:1],
            )
            scalar.activation(
                Ofc[:, 1, :], fps[1][:, :], mybir.ActivationFunctionType.Relu,
                bias=Bf[:, 1:2],
            ).then_inc(act_sem, 1)
            scalar.wait_ge(fc_sem, 2)
            scalar.copy(osb[:, :], dps[:, :]).then_inc(act_sem, 1)


    return nc


def _prep_weights(w_conv1, w_conv2, w_fc1, w_fc2, b_conv1, b_conv2, b_fc1, b_fc2):
    # wst [128, 2*NDR+2, 2, C] fp8: DR block (n, d) at [:, NDR*n + d];
    # blocks 14+n: (L|H)@tap2 fp8-only pair for the alternating 6-DR groups
    wst = np.zeros((128, 2 * NDR + 2, 2, C), dtype=NP_F8)
    for n, wc in enumerate((w_conv1, w_conv2)):
        w = np.asarray(wc[:, 0], np.float32) * WS          # [C, W, E]
        whi = w.astype(NP_F8).astype(np.float32)
        wlo = (w - whi).astype(NP_F8)                      # residual, same scale
        whi8 = w.astype(NP_F8)
        for t in range(W):
            for h, wsrc in enumerate((whi8, wlo)):
                bi = NDR * n + 2 * t + h
                wst[:, bi, 0, :] = wsrc[:, t, 0:128].T
                wst[:, bi, 1, :] = wsrc[:, t, 128:256].T
        # d=6: remainder mega + ovf, hi-only weights
        mega = np.zeros((128, C), dtype=NP_F8)
        mega[0:44] = whi8[:, 0, 256:300].T
        mega[44:88] = whi8[:, 1, 256:300].T
        mega[88:128] = whi8[:, 2, 256:296].T
        ovf = np.zeros((128, C), dtype=NP_F8)
        ovf[4 * n : 4 * n + 4] = whi8[:, 2, 296:300].T
        bi = NDR * n + 6
        wst[:, bi, 0, :] = mega              # ktile0 = mega_n
        wst[:, bi, 1, :] = ovf               # ktile1 = ovf plane (6)
        wst[:, 2 * NDR + n, 0, :] = whi8[:, 2, 0:128].T     # fp8 tap2 L
        wst[:, 2 * NDR + n, 1, :] = whi8[:, 2, 128:256].T   # fp8 tap2 H
    sc = np.float32(1.0 / (XS * WS))
    wfb = np.stack([w_fc1.T * sc, w_fc2.T * sc], axis=1).astype(NP_BF)  # [C, 2, O]
    bcs = np.stack([b_conv1, b_conv2], axis=1).astype(np.float32) * np.float32(XS * WS)
    bfs = np.stack([b_fc1, b_fc2], axis=1).astype(np.float32)
    return wst, wfb, bcs, bfs


def _prep_x_core(xq1, xq2):
    # xq1/xq2: fp8 [BS, L, E] for this core -> xp [7, 128, PADPOS]
    xs1 = np.ascontiguousarray(xq1.reshape(POS, E).T)      # [300, POS] fp8
    xs2 = np.ascontiguousarray(xq2.reshape(POS, E).T)
    xp = np.zeros((7, 128, PADPOS), dtype=NP_F8)
    xp[0, :, :POS] = xs1[0:128]
    xp[1, :, :POS] = xs1[128:256]
    xp[2, :, :POS] = xs2[0:128]
    xp[3, :, :POS] = xs2[128:256]
    for mi, xs in ((4, xs1), (5, xs2)):
        # mega rows: taps baked via pre-shift
        xp[mi, 0:44, :POS] = xs[256:300]
        xp[mi, 44:88, : POS - 1] = xs[256:300, 1:]
        xp[mi, 88:128, : POS - 2] = xs[256:296, 2:]
    xp[6, 0:4, : POS - 2] = xs1[296:300, 2:]
    xp[6, 4:8, : POS - 2] = xs2[296:300, 2:]
    return xp


def kernel(x1, x2, w_conv1, b_conv1, w_fc1, b_fc1, w_conv2, b_conv2, w_fc2, b_fc2):
    global LAST_RESULT
    xq1 = (np.asarray(x1, np.float32)[:, 0] * np.float32(XS)).astype(NP_F8)
    xq2 = (np.asarray(x2, np.float32)[:, 0] * np.float32(XS)).astype(NP_F8)
    wst, wfb, bcs, bfs = _prep_weights(
        np.asarray(w_conv1, np.float32), np.asarray(w_conv2, np.float32),
        np.asarray(w_fc1, np.float32), np.asarray(w_fc2, np.float32),
        np.asarray(b_conv1, np.float32), np.asarray(b_conv2, np.float32),
        np.asarray(b_fc1, np.float32), np.asarray(b_fc2, np.float32),
    )

    if "nc" not in _NC_CACHE:
        _NC_CACHE["nc"] = _build_nc()
    nc = _NC_CACHE["nc"]

    in_maps = []
    for c in range(NCORES):
        xp = _prep_x_core(xq1[c * BS : (c + 1) * BS], xq2[c * BS : (c + 1) * BS])
        in_maps.append({"xp": xp, "wst": wst, "wfb": wfb, "bc": bcs, "bf": bfs})

    res = run_bass_kernel_spmd(nc, in_maps, list(range(NCORES)), trace=TRACE)
    LAST_RESULT = res
    return np.concatenate(
        [res.results[c]["out"].reshape(BS, 1) for c in range(NCORES)], axis=0
    )
